# revision 1
# baseline (speedup 1.0000x reference)
"""Trainium2 Bass kernel for nn_ChebyshevEncoder.

Math (reference):
  xs = x * scale                                  [b, i]
  T_m = Chebyshev polynomials of xs, m = 0..7     [b, i, m]
  S[b,h,i,k] = sum_m T_m(xs[b,i]) * W[h,i,m,k],   W = kernels * poly  (folded on host)
  S = silu(S); flat to [b, f] with f = (h,i,k);  out = LayerNorm(flat) * gamma + beta

Device strategy (pure data parallel over batch, 8 cores, 512 rows each):
  - Chebyshev terms are re-expressed in a cheap power-ish basis
    B0=1, B1=xs, B2=2xs^2, B3=2xs^3, B4=2(B2-1)^2, B5=B2*B3, B6=2*B3^2, B7=B2*B5
    computed with 4 DVE multiplies + 3 ACT Square ops per batch tile; the
    exact T->B change of basis is folded into the weights on the host (f64).
  - The per-feature contraction over m becomes a block-diagonal matmul:
    for each 16-feature chunk, K = 8 basis fns x 16 features = 128.
    Basis tiles are transposed on the TensorE (PE) into PSUM, cast to bf16,
    and used as the stationary operand; weights stream as bf16.
  - PSUM banks accumulate (h, i64-group) output blocks; ScalarE drains them
    with fused Silu + per-row accumulation (running sum for LayerNorm mean).
  - Sum of squares via one tensor_tensor_reduce pass; normalize + gamma via
    fused DVE ops on bf16 (2x/4x perf modes); beta on GPSIMD if nonzero.
  - Output written as bf16 and upcast to f32 on the host (error well under
    1% of absmax; validated against the f64 reference).
"""

import os

import numpy as np
import ml_dtypes

BATCH = 4096
INPUT = 512
MAX_TERMS = 8
HEADS = 4
KSIZE = 8
F = HEADS * INPUT * KSIZE  # 16384
N_CORES = 8
ROWS = BATCH // N_CORES    # 512 rows per core
P = 128
NBT = ROWS // P            # 4 batch tiles per core
NCHUNK = INPUT // 16       # 32 feature chunks (16 features each)
LN_EPS = 1e-5

# basis block order in the TT buffer: [B1..B7, ones(B0)]
_BMAP = [1, 2, 3, 4, 5, 6, 7, 0]

# B basis in monomial coeffs (index = degree)
_BPOLYS = [
    [1],
    [0, 1],
    [0, 0, 2],
    [0, 0, 0, 2],
    [2, 0, -8, 0, 8],
    [0, 0, 0, 0, 0, 4],
    [0, 0, 0, 0, 0, 0, 8],
    [0, 0, 0, 0, 0, 0, 0, 8],
]

_NC_CACHE = {}
_LAST_EXEC_NS = {}
_ACT_FN = "Silu"  # debug knob: CoreSim lacks Silu; tests may set "Sigmoid"


def _cheb_to_b_matrix():
    """C with T_m = sum_mp C[m, mp] * B_mp (exact, small ints)."""
    bmat = np.zeros((8, 8))
    for i, p in enumerate(_BPOLYS):
        bmat[i, : len(p)] = p
    tmat = np.zeros((8, 8))
    for m in range(8):
        c = np.zeros(8)
        c[m] = 1
        tmat[m, : m + 1] = np.polynomial.chebyshev.cheb2poly(c)
    C = np.linalg.solve(bmat.T, tmat.T).T
    assert np.abs(C @ bmat - tmat).max() < 1e-9
    return C


def _build_weights(poly_weights, kernels):
    """Fold poly into kernels, change basis, lay out as [chunk, K=128, N=512] bf16.

    K rows: m_blk*16 + i16 with basis order _BMAP; N cols: h*128 + i16*8 + k.
    """
    W = kernels.astype(np.float64) * poly_weights.astype(np.float64)[:, :, None, :]
    C = _cheb_to_b_matrix()
    WB = np.einsum("himk,mn->nhik", W, C)          # [8(mp), H, I, K]
    WBr = WB[_BMAP].reshape(8, HEADS, NCHUNK, 16, KSIZE)  # [m_blk, h, c, i16, k]
    Wdev = np.zeros((NCHUNK, 8, 16, HEADS, 16, KSIZE), np.float64)
    ii = np.arange(16)
    # Wdev[c, m_blk, i, h, i, k] = WBr[m_blk, h, c, i, k]
    # advanced indices (positions 2 and 4) land in front: LHS view is [16, c, 8, h, k]
    Wdev[:, :, ii, :, ii, :] = np.transpose(WBr, (3, 2, 0, 1, 4))
    Wdev = Wdev.reshape(NCHUNK, 128, 512)
    # SBUF layout: [partition K=128, chunk-major free] so the DMA is contiguous
    Wdev = np.ascontiguousarray(Wdev.transpose(1, 0, 2).reshape(128, NCHUNK * 512))
    return Wdev.astype(ml_dtypes.bfloat16)


def _build_nc(apply_gamma, apply_beta):
    from concourse import bacc
    import concourse.mybir as mybir
    from concourse.tile import TileContext

    dt = mybir.dt
    AF = mybir.ActivationFunctionType
    OP = mybir.AluOpType
    SQ2 = float(np.sqrt(2.0))

    nc = bacc.Bacc(None, target_bir_lowering=False)

    x_d = nc.dram_tensor("x", [ROWS, INPUT], dt.float32, kind="ExternalInput")
    w_d = nc.dram_tensor("wb", [P, NCHUNK * 512], dt.bfloat16, kind="ExternalInput")
    sc_d = nc.dram_tensor("scale_bc", [P, INPUT], dt.float32, kind="ExternalInput")
    id_d = nc.dram_tensor("ident", [P, P], dt.float32, kind="ExternalInput")
    g_d = b_d = None
    if apply_gamma:
        g_d = nc.dram_tensor("gamma_bc", [P, F], dt.bfloat16, kind="ExternalInput")
    if apply_beta:
        b_d = nc.dram_tensor("beta_bc", [P, F], dt.bfloat16, kind="ExternalInput")
    y_d = nc.dram_tensor("y", [ROWS, F], dt.bfloat16, kind="ExternalOutput")

    with TileContext(nc) as tc:
        with (
            tc.tile_pool(name="const", bufs=1) as constp,
            tc.tile_pool(name="xin", bufs=1) as xinp,
            tc.tile_pool(name="tt", bufs=NBT) as ttp,
            tc.tile_pool(name="lq", bufs=6) as lqp,
            tc.tile_pool(name="sbig", bufs=2) as sp,
            tc.tile_pool(name="sqd", bufs=1) as sqdp,
            tc.tile_pool(name="stats", bufs=2) as stp,
            tc.tile_pool(name="mm", bufs=3, space="PSUM") as mmp,
            tc.tile_pool(name="tr", bufs=2, space="PSUM") as trp,
        ):
            # urgent small inputs on the SP ring; big weights on the ACT ring
            x_sb = xinp.tile([P, NBT * INPUT], dt.float32)
            nc.sync.dma_start(
                out=x_sb.rearrange("p (t i) -> p t i", t=NBT),
                in_=x_d.rearrange("(t p) i -> p t i", p=P),
            )
            sc_sb = constp.tile([P, INPUT], dt.float32)
            nc.sync.dma_start(out=sc_sb[:], in_=sc_d[:])
            id_sb = constp.tile([P, P], dt.float32)
            nc.sync.dma_start(out=id_sb[:], in_=id_d[:])
            w_sb = constp.tile([P, NCHUNK * 512], dt.bfloat16)
            nc.scalar.dma_start(out=w_sb[:], in_=w_d[:])
            if apply_gamma:
                g_sb = constp.tile([P, F], dt.bfloat16)
                nc.scalar.dma_start(out=g_sb[:], in_=g_d[:])
            if apply_beta:
                b_sb = constp.tile([P, F], dt.bfloat16)
                nc.scalar.dma_start(out=b_sb[:], in_=b_d[:])

            # quake-rsqrt integer constants
            magic = constp.tile([P, 1], dt.int32)
            nc.vector.memset(magic[:], 0x5F3759DF)
            shift1 = constp.tile([P, 1], dt.int32)
            nc.vector.memset(shift1[:], 1)

            sqd = sqdp.tile([P, 2048], dt.bfloat16)  # sq-pass scratch output

            v = nc.vector
            a = nc.scalar

            # ---- basis for all batch tiles upfront (keeps PE stream dense) ----
            tts = []
            for bt in range(NBT):
                xt = x_sb[:, bt * INPUT : (bt + 1) * INPUT]
                tt = ttp.tile([P, 8 * INPUT], dt.float32)
                tts.append(tt)
                # layout: col = c*128 + m_blk*16 + i16
                t4 = tt.rearrange("p (c m i) -> p c m i", c=NCHUNK, m=8)
                x3 = xt.rearrange("p (c i) -> p c i", c=NCHUNK)
                s3 = sc_sb.rearrange("p (c i) -> p c i", c=NCHUNK)
                v.tensor_tensor(t4[:, :, 0], x3, s3, OP.mult)                    # B1 = xs
                v.scalar_tensor_tensor(t4[:, :, 1], t4[:, :, 0], 2.0, t4[:, :, 0], OP.mult, OP.mult)  # B2
                v.tensor_tensor(t4[:, :, 2], t4[:, :, 0], t4[:, :, 1], OP.mult)  # B3
                v.tensor_scalar(t4[:, :, 3], t4[:, :, 1], -1.0, None, OP.add)    # B2-1
                v.scalar_tensor_tensor(t4[:, :, 3], t4[:, :, 3], 2.0, t4[:, :, 3], OP.mult, OP.mult)  # B4
                v.tensor_tensor(t4[:, :, 4], t4[:, :, 1], t4[:, :, 2], OP.mult)  # B5
                v.scalar_tensor_tensor(t4[:, :, 5], t4[:, :, 2], 2.0, t4[:, :, 2], OP.mult, OP.mult)  # B6
                v.tensor_tensor(t4[:, :, 6], t4[:, :, 1], t4[:, :, 4], OP.mult)  # B7
                v.memset(t4[:, :, 7], 1.0)                                       # ones

            act_fn = getattr(AF, _ACT_FN)
            for bt in range(NBT):
                tt = tts[bt]
                s_t = sp.tile([P, F], dt.bfloat16)
                # f = h*4096 + (ig*64 + j*16 + i16)*8 + k
                s6 = s_t.rearrange(
                    "p (h g j i k) -> p h g j i k", h=HEADS, g=8, j=4, i=16
                )
                strip = stp.tile([P, 32], dt.float32, tag="strip")

                for ig in range(8):
                    trq = trp.tile([P, 512], dt.float32, space="PSUM")
                    for j in range(4):
                        c = 4 * ig + j
                        nc.tensor.transpose(
                            trq[:, j * P : (j + 1) * P],
                            tt[:, c * P : (c + 1) * P],
                            id_sb[:],
                        )
                    lq = lqp.tile([P, 512], dt.bfloat16)
                    a.copy(lq[:], trq[:])  # cast f32 -> bf16

                    mmA = mmp.tile([P, 1024], dt.float32, space="PSUM", tag="mm")
                    mmB = mmp.tile([P, 1024], dt.float32, space="PSUM", tag="mm")
                    for j in range(4):
                        c = 4 * ig + j
                        dst = (mmA if j < 2 else mmB)[:, (j % 2) * 512 : (j % 2 + 1) * 512]
                        nc.tensor.matmul(
                            dst,
                            lq[:, j * P : (j + 1) * P],
                            w_sb[:, c * 512 : (c + 1) * 512],
                            start=True,
                            stop=True,
                        )
                    # silu drains with running row-sum; psum bank j holds
                    # (h, i16, k) for chunk c = 4*ig + j
                    a.activation(
                        s6[:, :, ig, 0:2, :, :],
                        mmA.rearrange("p (j h i k) -> p h j i k", j=2, h=HEADS, i=16),
                        act_fn,
                        accum_out=strip[:, 2 * ig : 2 * ig + 1],
                    )
                    a.activation(
                        s6[:, :, ig, 2:4, :, :],
                        mmB.rearrange("p (j h i k) -> p h j i k", j=2, h=HEADS, i=16),
                        act_fn,
                        accum_out=strip[:, 2 * ig + 1 : 2 * ig + 2],
                    )
                    # sum-of-squares for this ig (overlaps later drains)
                    sview = s6[:, :, ig, :, :, :]
                    v.scalar_tensor_tensor(
                        sqd.rearrange("p (h j i k) -> p h j i k", h=HEADS, j=4, i=16),
                        sview,
                        1.0,
                        sview,
                        OP.mult,
                        OP.mult,
                        accum_out=strip[:, 16 + ig : 17 + ig],
                    )

                # ---- layernorm stats ----
                st = stp.tile([P, 16], dt.float32, tag="st")
                sti = st.bitcast(dt.int32)
                rowsum = st[:, 0:1]
                v.tensor_reduce(rowsum, strip[:, 0:16], mybir.AxisListType.X, OP.add)
                sumsq = st[:, 1:2]
                v.tensor_reduce(sumsq, strip[:, 16:24], mybir.AxisListType.X, OP.add)
                mean = st[:, 2:3]
                v.tensor_scalar(mean, rowsum, 1.0 / F, None, OP.mult)
                ex2 = st[:, 3:4]
                v.tensor_scalar(ex2, sumsq, 1.0 / F, None, OP.mult)
                nm2 = st[:, 4:5]
                v.tensor_scalar(nm2, mean, mean, -1.0, OP.mult, OP.mult)
                vpe = st[:, 5:6]
                v.scalar_tensor_tensor(vpe, ex2, LN_EPS, nm2, OP.add, OP.add)
                # quake rsqrt + 3 Newton steps (all DVE; keeps ACT table on Silu)
                bits = sti[:, 6:7]
                v.tensor_scalar(bits, sti[:, 5:6], shift1[:, 0:1], None, OP.arith_shift_right)
                r0i = sti[:, 7:8]
                v.tensor_tensor(r0i, magic[:, 0:1], bits, OP.subtract)
                r = st[:, 7:8]  # same bytes as r0i, viewed f32
                for it in range(3):
                    m1 = st[:, 8 + 2 * it : 9 + 2 * it]
                    v.tensor_tensor(m1, r, r, OP.mult)
                    m2 = st[:, 9 + 2 * it : 10 + 2 * it]
                    v.tensor_tensor(m2, m1, vpe, OP.mult)
                    v.tensor_scalar(m2, m2, -0.5, 1.5, OP.mult, OP.add)
                    rn = st[:, 14:15] if it == 2 else st[:, 8 + 2 * it : 9 + 2 * it]
                    v.tensor_tensor(rn, r, m2, OP.mult)
                    r = rn
                rstd = r
                biasp = st[:, 15:16]
                v.tensor_scalar(biasp, mean, rstd, -1.0, OP.mult, OP.mult)

                # ---- normalize (+ gamma/beta) in place ----
                v.tensor_scalar(s_t[:], s_t[:], rstd, biasp, OP.mult, OP.add)
                if apply_gamma:
                    v.tensor_tensor(s_t[:], s_t[:], g_sb[:], OP.mult)
                if apply_beta:
                    v.tensor_tensor(s_t[:], s_t[:], b_sb[:], OP.add)

                nc.sync.dma_start(out=y_d[bt * P : (bt + 1) * P, :], in_=s_t[:])

    nc.compile()
    return nc


def _get_nc(apply_gamma, apply_beta):
    key = (apply_gamma, apply_beta)
    if key not in _NC_CACHE:
        _NC_CACHE[key] = _build_nc(apply_gamma, apply_beta)
    return _NC_CACHE[key]


def _install_axon_ntff_hook():
    """Benchmark-only: provide antenv.axon_hooks if the image lacks it, so
    run_bass_kernel_spmd(trace=True) can capture NTFF profiles under axon."""
    import sys
    import types
    import ctypes
    import contextlib

    try:
        from antenv.axon_hooks import get_axon_ntff_profile_hook  # noqa: F401

        return
    except ImportError:
        pass
    so_path = os.environ.get("PJRT_LIBRARY_PATH", "/opt/axon/libaxon_pjrt.so")
    try:
        lib = ctypes.CDLL(so_path)
    except OSError:
        return
    if not hasattr(lib, "axon_start_nrt_profile"):
        return
    lib.axon_start_nrt_profile.argtypes = [
        ctypes.POINTER(ctypes.c_int64),
        ctypes.c_size_t,
    ]
    lib.axon_start_nrt_profile.restype = ctypes.c_int64
    lib.axon_stop_nrt_profile.argtypes = [ctypes.c_char_p]
    lib.axon_stop_nrt_profile.restype = ctypes.c_int64

    @contextlib.contextmanager
    def _hook(output_dir, device_ids):
        import jax

        jax.devices()
        if device_ids:
            ids = (ctypes.c_int64 * len(device_ids))(*device_ids)
            rc = lib.axon_start_nrt_profile(ids, len(device_ids))
        else:
            rc = lib.axon_start_nrt_profile(None, 0)
        if rc != 0:
            raise RuntimeError(f"axon_start_nrt_profile rc={rc}")
        try:
            yield
        finally:
            n = lib.axon_stop_nrt_profile(str(output_dir).encode())
            print(f"ntff profile: {n} file(s) written to {output_dir}")

    mod = types.ModuleType("antenv.axon_hooks")
    mod.get_axon_ntff_profile_hook = lambda: _hook
    mod.set_axon_ntff_profile_hook = lambda h: None
    sys.modules["antenv.axon_hooks"] = mod
    import antenv

    antenv.axon_hooks = mod


def kernel(x, scale_param, poly_weights, kernels, ln_gamma, ln_beta):
    from concourse.bass_utils import run_bass_kernel_spmd

    x = np.asarray(x, dtype=np.float32)
    scale_param = np.asarray(scale_param, dtype=np.float32)
    poly_weights = np.asarray(poly_weights, dtype=np.float32)
    kernels = np.asarray(kernels, dtype=np.float32)
    ln_gamma = np.asarray(ln_gamma, dtype=np.float32)
    ln_beta = np.asarray(ln_beta, dtype=np.float32)

    apply_gamma = not np.all(ln_gamma == 1.0)
    apply_beta = not np.all(ln_beta == 0.0)

    wdev = _build_weights(poly_weights, kernels)
    sc_bc = np.ascontiguousarray(np.broadcast_to(scale_param[None, :], (P, INPUT))).astype(
        np.float32
    )
    ident = np.eye(P, dtype=np.float32)

    base = {
        "wb": wdev,
        "scale_bc": sc_bc,
        "ident": ident,
    }
    if apply_gamma:
        base["gamma_bc"] = np.ascontiguousarray(
            np.broadcast_to(ln_gamma[None, :], (P, F))
        ).astype(ml_dtypes.bfloat16)
    if apply_beta:
        base["beta_bc"] = np.ascontiguousarray(
            np.broadcast_to(ln_beta[None, :], (P, F))
        ).astype(ml_dtypes.bfloat16)

    in_maps = []
    for core in range(N_CORES):
        m = dict(base)
        m["x"] = np.ascontiguousarray(x[core * ROWS : (core + 1) * ROWS])
        in_maps.append(m)

    nc = _get_nc(apply_gamma, apply_beta)

    trace = os.environ.get("KBENCH_TRACE", "0") == "1"
    if trace:
        _install_axon_ntff_hook()
    res = run_bass_kernel_spmd(
        nc,
        in_maps,
        core_ids=list(range(N_CORES)),
        trace=trace,
    )
    _LAST_EXEC_NS["exec_time_ns"] = res.exec_time_ns
    _LAST_EXEC_NS["trace"] = res.instructions_and_trace[1] if res.instructions_and_trace else None

    out = np.concatenate([r["y"] for r in res.results], axis=0)
    return out.astype(np.float32)

